# revision 39
# baseline (speedup 1.0000x reference)
"""Trainium2 Bass kernel for a dense transformer block (B=2, T=2048, C=1024, H=16).

Sharding: tensor-parallel attention (2 heads/core, processed as two head
passes each followed by an AllToAll so the first collective hides under the
second pass) + row-parallel Wo/FFN (512 rows/core) across 8 NeuronCores.

Big GEMMs (QKV / Wo / W1 / W2) run fp8e4 with DoubleRow perf mode (2
k-rows/partition). Weights are pre-scaled by 16x on the host so their
uniform(-1/32,1/32) entries land in e4m3 normal range; the scale is undone
via the softmax exp scale, the V "ones" column, and a final /16 on the host
(every correction is a power of two, so exact).

Softmax denominators come free from PV matmuls via a constant column
appended to V; the reciprocal runs as a fast custom-DVE op and is broadcast
across partitions on the (otherwise idle) GPSIMD engine. LayerNorm rstd uses
exp(-0.5*ln(var+eps)) so the whole kernel needs a single activation table
set (natural_log_exp_and_others).
"""

import numpy as np
import ml_dtypes

import concourse.bass as bass
import concourse.bacc as bacc
import concourse.mybir as mybir
import concourse.tile as tile
from concourse.masks import make_identity


F32 = mybir.dt.float32
BF16 = mybir.dt.bfloat16
F8 = mybir.dt.float8e4
AF = mybir.ActivationFunctionType
ALU = mybir.AluOpType
DR = mybir.MatmulPerfMode.DoubleRow

N_CORES = 8
B, T, C, H, D, FF = 2, 2048, 1024, 16, 64, 4096
R = B * T            # 4096 total rows
RS = R // N_CORES    # 512 rows per core
KT = C // 128        # 8 k-tiles of the embedding dim
SCALE = 1.0 / np.sqrt(C)     # 2**-5 exact
EXP_SCALE = SCALE / 256.0    # undo the 16x on both Wq and Wk
LN_EPS = 1e-5
DEN_CONST = 16.0     # V "ones" column value -> pa[64] = 16*den -> rec = 1/(16*den)
OUT_SCALE = 1.0 / 16.0   # device output is 16*y; undone on the host


def build_nc():
    nc = bacc.Bacc(None, target_bir_lowering=False, debug=False, num_devices=N_CORES)

    # ---- per-core inputs (host pre-laid-out) ----
    x_bf = nc.dram_tensor("x_bf", [8, 128, 4 * C], BF16, kind="ExternalInput").ap()
    xs16 = nc.dram_tensor("xs16", [4, 128, C], F32, kind="ExternalInput").ap()
    wq = nc.dram_tensor("wq", [128, KT, 128], F8, kind="ExternalInput").ap()
    wk = nc.dram_tensor("wk", [128, KT, 128], F8, kind="ExternalInput").ap()
    wv = nc.dram_tensor("wv", [128, KT, 128], F8, kind="ExternalInput").ap()
    wo = nc.dram_tensor("wo", [128, KT, C], BF16, kind="ExternalInput").ap()
    w1 = nc.dram_tensor("w1", [128, KT, FF], F8, kind="ExternalInput").ap()
    b1 = nc.dram_tensor("b1", [128, 32], F32, kind="ExternalInput").ap()
    w2 = nc.dram_tensor("w2", [128, FF // 128, C], F8, kind="ExternalInput").ap()
    b2row = nc.dram_tensor("b2row", [1, C], BF16, kind="ExternalInput").ap()
    masks = nc.dram_tensor("masks", [4, 128, 512], BF16, kind="ExternalInput").ap()
    y = nc.dram_tensor("y", [4, 128, C], F32, kind="ExternalOutput").ap()
    import os
    dbg = os.environ.get("KDBG") == "1"
    if dbg:
        dq = nc.dram_tensor("dq", [128, R], BF16, kind="ExternalOutput").ap()
        dk = nc.dram_tensor("dk", [128, R], BF16, kind="ExternalOutput").ap()
        dv = nc.dram_tensor("dv", [128, R], BF16, kind="ExternalOutput").ap()
        dat = nc.dram_tensor("dat", [N_CORES, 64, RS], BF16, kind="ExternalOutput").ap()
        dattnt = nc.dram_tensor("dattnt", [128, KT, RS], BF16,
                                kind="ExternalOutput").ap()
        dx2 = nc.dram_tensor("dx2", [128, 4, C], F32, kind="ExternalOutput").ap()
        dh1 = nc.dram_tensor("dh1", [128, KT, 512], F8, kind="ExternalOutput").ap()
        dpa = nc.dram_tensor("dpa", [65, 512], F32, kind="ExternalOutput").ap()
        drec = nc.dram_tensor("drec", [1, 512], F32, kind="ExternalOutput").ap()
        drecb = nc.dram_tensor("drecb", [64, 512], F32, kind="ExternalOutput").ap()
        dpt = nc.dram_tensor("dpt", [128, 1024], BF16, kind="ExternalOutput").ap()

    with tile.TileContext(nc) as tc:
        with (
            tc.tile_pool(name="const", bufs=1) as const,
            tc.tile_pool(name="ps_mm", bufs=2, space="PSUM") as ps_mm,
            tc.tile_pool(name="ps_s", bufs=2, space="PSUM") as ps_s,
            tc.tile_pool(name="ps_a", bufs=2, space="PSUM") as ps_a,
            tc.tile_pool(name="dram", bufs=1, space="DRAM") as dram,
        ):
            # constants
            epst = const.tile([128, 1], F32)
            nc.any.memset(epst[:], LN_EPS)
            mask_sb = const.tile([128, 4, 512], BF16)
            for d in range(4):
                nc.scalar.dma_start(mask_sb[:, d, :], masks[d])
            wq_sb = const.tile([128, KT, 128], F8)
            nc.scalar.dma_start(wq_sb[:], wq[:])
            wk_sb = const.tile([128, KT, 128], F8)
            nc.scalar.dma_start(wk_sb[:], wk[:])
            wv_sb = const.tile([128, KT, 128], F8)
            nc.scalar.dma_start(wv_sb[:], wv[:])
            b1_sb = const.tile([128, 32], F32)
            nc.scalar.dma_start(b1_sb[:], b1[:])
            b2row_sb = const.tile([1, C], BF16)
            nc.scalar.dma_start(b2row_sb[:], b2row[:])
            ones1x128 = const.tile([1, 128], BF16)
            nc.any.memset(ones1x128[:], 1.0)
            ones64b = const.tile([1, 64], BF16)
            nc.any.memset(ones64b[:], 1.0)
            ident = const.tile([128, 128], BF16)
            make_identity(nc, ident[:])
            # persistent stage E/F tensors (prefetched at chosen points)
            wo_sb = const.tile([128, KT, C], BF16)
            w2t = const.tile([128, 32, C], F8)
            w1t = const.tile([128, KT, FF], F8)
            xs_sb = const.tile([128, 4, C], F32)
            attnt = const.tile([128, KT, RS], BF16)

            # prime the exp activation table set during warmup
            lnprime = const.tile([128, 1], F32)
            nc.scalar.activation(out=lnprime[:], in_=epst[:], func=AF.Exp)
            # magic constant for quake-rsqrt on the DVE (no ACT table needed)
            U32 = mybir.dt.uint32
            magic4 = const.tile([128, 4], F32)
            nc.any.memset(magic4[:].bitcast(U32), 0x5F3759DF)

            def emit_rsqrt(pool, var_ap, ncols, tagp):
                """rstd = 1/sqrt(var_ap + eps) via quake seed + 2 Newton steps."""
                ve = pool.tile([128, ncols], F32, tag=tagp + "ve")
                nc.vector.tensor_scalar(out=ve[:], in0=var_ap, scalar1=LN_EPS,
                                        scalar2=None, op0=ALU.add)
                y0 = pool.tile([128, ncols], F32, tag=tagp + "y0")
                nc.vector.tensor_scalar(out=y0[:].bitcast(U32),
                                        in0=ve[:].bitcast(U32), scalar1=1,
                                        scalar2=None,
                                        op0=ALU.logical_shift_right)
                nc.vector.tensor_tensor(out=y0[:].bitcast(U32),
                                        in0=magic4[:, 0:ncols].bitcast(U32),
                                        in1=y0[:].bitcast(U32), op=ALU.subtract)
                t = pool.tile([128, ncols], F32, tag=tagp + "tq")
                for _ in range(2):
                    nc.vector.tensor_tensor(out=t[:], in0=ve[:], in1=y0[:],
                                            op=ALU.mult)
                    nc.vector.tensor_tensor(out=t[:], in0=t[:], in1=y0[:],
                                            op=ALU.mult)
                    nc.vector.tensor_scalar(out=t[:], in0=t[:], scalar1=-0.5,
                                            scalar2=1.5, op0=ALU.mult,
                                            op1=ALU.add)
                    nc.vector.tensor_tensor(out=y0[:], in0=y0[:], in1=t[:],
                                            op=ALU.mult)
                return y0

            # HAM warmup: PE activity with no DMA dependency
            for wi in range(88):
                ps_w = ps_mm.tile([128, 512], F32, tag="psmm")
                nc.tensor.matmul(ps_w[:, 0:128], lhsT=ident[:], rhs=ident[:],
                                 start=True, stop=True)


            a2a_in = [dram.tile([N_CORES, 64, RS], BF16, name=f"a2ain{h}")
                      for h in range(2)]
            a2a_out = [dram.tile([N_CORES, 64, RS], BF16, name=f"a2aout{h}")
                       for h in range(2)]
            # early barrier: absorb per-core launch skew before the real work
            bar_in = dram.tile([N_CORES, 1, 16], F32, name="barin")
            bar_out = dram.tile([N_CORES, 1, 16], F32, name="barout")
            barsrc = const.tile([1, 16], F32)
            nc.any.memset(barsrc[:], 0.0)
            for s in range(N_CORES):
                nc.sync.dma_start(out=bar_in[s], in_=barsrc[:])
            nc.gpsimd.collective_compute(
                "AllToAll", ALU.bypass,
                replica_groups=[list(range(N_CORES))],
                ins=[bar_in[:].opt()], outs=[bar_out[:].opt()],
            )

            # attention-stage persistent tiles
            attn_pool_cm = tc.tile_pool(name="attn", bufs=1)
            attn = attn_pool_cm.__enter__()
            qt_sb = attn.tile([128, R], BF16)   # Q^T, feature-major (2 heads stacked)
            kt_sb = attn.tile([128, R], BF16)   # K^T
            vt_sb = attn.tile([128, R], BF16)   # V^T (pre-transpose)
            # V token-major, per 128-token chunk: [h0 d0..63 | 16.0 | h1 d0..63 | 16.0]
            v_sb = attn.tile([128, 32, 130], BF16)
            nc.vector.memset(v_sb[:, :, 64:65], DEN_CONST)
            nc.vector.memset(v_sb[:, :, 129:130], DEN_CONST)
            ptp_cm = tc.tile_pool(name="ptp", bufs=5)
            ptp = ptp_cm.__enter__()
            smp_cm = tc.tile_pool(name="smp", bufs=2)
            smp = smp_cm.__enter__()

            # ====== Stage A+B: LN1 + transpose + QKV per 512-row chunk ======
            with (
                tc.tile_pool(name="lnp", bufs=4) as lnp,
                tc.tile_pool(name="h1tp", bufs=2) as h1tp,
            ):
                for n in range(R // 512):
                    h1tn = h1tp.tile([128, KT, 512], F8, tag="h1tn")
                    xt4 = lnp.tile([128, 4 * C], BF16, tag="xt4", bufs=2)
                    if n == 0:
                        for i4 in range(4):
                            qeng = nc.sync if i4 % 2 == 0 else nc.scalar
                            qeng.dma_start(xt4[:, C * i4:C * (i4 + 1)],
                                           x_bf[n][:, C * i4:C * (i4 + 1)])
                    else:
                        qeng = nc.sync if n % 2 == 0 else nc.scalar
                        qeng.dma_start(xt4[:], x_bf[n])
                    mvn = lnp.tile([128, 4, 2], F32, tag="mvn")
                    xg = xt4[:].rearrange("p (i s f) -> p i s f", i=4, f=512)
                    for i4 in range(4):
                        stats = lnp.tile([128, 2, 6], F32, tag="st")
                        for s in range(2):
                            nc.vector.bn_stats(out=stats[:, s, :],
                                               in_=xg[:, i4, s, :])
                        nc.vector.bn_aggr(out=mvn[:, i4, :], in_=stats[:])
                    rstd4 = emit_rsqrt(lnp, mvn[:, :, 1], 4, "a")
                    for i4 in range(4):
                        hn = lnp.tile([128, C], BF16, tag="hn")
                        nc.vector.tensor_scalar(out=hn[:],
                                                in0=xt4[:, C * i4:C * (i4 + 1)],
                                                scalar1=mvn[:, i4, 0:1],
                                                scalar2=rstd4[:, i4:i4 + 1],
                                                op0=ALU.subtract, op1=ALU.mult)
                        for half in range(2):
                            ps_t = ps_mm.tile([128, 512], BF16, tag="psmm")
                            for j4 in range(4):
                                j = 4 * half + j4
                                nc.tensor.transpose(ps_t[:, 128 * j4:128 * (j4 + 1)],
                                                    hn[:, 128 * j:128 * (j + 1)],
                                                    ident[:])
                            nc.vector.tensor_copy(
                                out=h1tn[:, 4 * half:4 * half + 4,
                                         128 * i4:128 * (i4 + 1)],
                                in_=ps_t[:].rearrange("p (a b) -> p a b", a=4))
                    for w_sb, out_sb in ((wq_sb, qt_sb), (wk_sb, kt_sb),
                                         (wv_sb, vt_sb)):
                        ps = ps_mm.tile([128, 512], F32, tag="psmm")
                        for t in range(KT // 2):
                            nc.tensor.matmul(ps[:], lhsT=w_sb[:, 2 * t:2 * t + 2, :],
                                             rhs=h1tn[:, 2 * t:2 * t + 2, :],
                                             start=(t == 0), stop=(t == KT // 2 - 1),
                                             perf_mode=DR)
                        nc.scalar.copy(out=out_sb[:, 512 * n:512 * (n + 1)],
                                       in_=ps[:])
                    if dbg and n == 0:
                        nc.sync.dma_start(dh1[:], h1tn[:])
                    # V token-major for this 512-token group (PE transposes)
                    g = n
                    ps_t = ps_mm.tile([128, 512], BF16, tag="psmm")
                    for j4 in range(4):
                        j = 4 * g + j4
                        nc.tensor.transpose(ps_t[:, 128 * j4:128 * (j4 + 1)],
                                            vt_sb[:, 128 * j:128 * (j + 1)], ident[:])
                    pst3 = ps_t[:].rearrange("p (a b) -> p a b", a=4)
                    nc.scalar.copy(out=v_sb[:, 4 * g:4 * g + 4, 0:64],
                                   in_=pst3[:, :, 0:64])
                    nc.scalar.copy(out=v_sb[:, 4 * g:4 * g + 4, 65:129],
                                   in_=pst3[:, :, 64:128])

            # stage-E inputs (xs16 2MB + wo 2MB) now that x has drained
            for j in range(4):
                nc.scalar.dma_start(xs_sb[:, j, :], xs16[j])
            nc.scalar.dma_start(wo_sb[:], wo[:])
            if dbg:
                nc.sync.dma_start(dq[:], qt_sb[:])
                nc.sync.dma_start(dk[:], kt_sb[:])
                nc.sync.dma_start(dv[:], vt_sb[:])

            # =============== Stage C: attention, one head per pass ===============
            def finish_tail(pend):
                anum, rec, hh, shard = pend
                pb = ps_mm.tile([64, 512], F32, tag="psmm")
                nc.tensor.matmul(pb[:], lhsT=ones64b[:], rhs=rec[:],
                                 start=True, stop=True)
                at16 = smp.tile([64, 512], BF16, tag="at16")
                with nc.allow_low_precision(reason="attn out bf16"):
                    nc.vector.tensor_tensor(out=at16[:], in0=anum[:], in1=pb[:],
                                            op=ALU.mult)
                nc.sync.dma_start(out=a2a_in[hh][shard], in_=at16[:])

            pend = None
            for h in range(2):
                hp = 64 * h
                vo = 65 * h
                for b in range(B):
                    for qc in range(4):
                        q0 = b * T + 512 * qc
                        nkt = 4 * (qc + 1)
                        npair = nkt // 2
                        pts = {}
                        pa = ps_a.tile([65, 512], F32, tag="pa")

                        def emit_qk(p):
                            ps = ps_s.tile([128, 1024], F32, tag="pss")
                            for u in range(2):
                                k = 2 * p + u
                                nc.tensor.matmul(
                                    ps[:, 512 * u:512 * (u + 1)],
                                    lhsT=kt_sb[hp:hp + 64,
                                               b * T + 128 * k:b * T + 128 * (k + 1)],
                                    rhs=qt_sb[hp:hp + 64, q0:q0 + 512],
                                    start=True, stop=True, tile_position=(hp, 0))
                            pt = ptp.tile([128, 1024], BF16, tag="pt")
                            nc.scalar.activation(out=pt[:], in_=ps[:],
                                                 func=AF.Exp, scale=EXP_SCALE)
                            for u in range(2):
                                k = 2 * p + u
                                if k >= 4 * qc:
                                    sl = pt[:, 512 * u:512 * (u + 1)]
                                    nc.vector.tensor_tensor(
                                        out=sl, in0=sl,
                                        in1=mask_sb[:, k - 4 * qc, :], op=ALU.mult)
                            pts[p] = pt

                        def emit_pv(p):
                            for u in range(2):
                                k = 2 * p + u
                                nc.tensor.matmul(
                                    pa[:],
                                    lhsT=v_sb[:, b * 16 + k, vo:vo + 65],
                                    rhs=pts[p][:, 512 * u:512 * (u + 1)],
                                    start=(k == 0), stop=(k == nkt - 1),
                                    tile_position=(0, 0))

                        for p in range(0, npair, 2):
                            emit_qk(p)
                            emit_qk(p + 1)
                            if p >= 2:
                                emit_pv(p - 2)
                                emit_pv(p - 1)
                        emit_pv(npair - 2)
                        emit_pv(npair - 1)
                        # softmax tail: anum + den to SBUF (frees pa),
                        # rec = 1/(16*den); broadcast matmul deferred one chunk
                        anum = smp.tile([64, 512], BF16, tag="anum")
                        with nc.allow_low_precision(reason="attn numerator bf16"):
                            nc.vector.tensor_copy(out=anum[:], in_=pa[0:64, :])
                        dcp = smp.tile([1, 512], F32, tag="dcp")
                        nc.scalar.copy(out=dcp[:], in_=pa[64:65, :])
                        rec32 = smp.tile([1, 512], F32, tag="rec32")
                        rscr = smp.tile([1, 512], F32, tag="rscr")
                        nc.vector.reciprocal_approx_accurate(out=rec32[:],
                                                             in_=dcp[:],
                                                             scratch=rscr[:])
                        rec = smp.tile([1, 512], BF16, tag="rec")
                        with nc.allow_low_precision(reason="softmax denom bf16"):
                            nc.vector.tensor_copy(out=rec[:], in_=rec32[:])
                        if dbg and h == 0 and b == 0 and qc == 0:
                            pacp = smp.tile([65, 512], F32, tag="pacp")
                            nc.scalar.copy(out=pacp[:], in_=pa[:])
                            nc.sync.dma_start(out=dpa[:], in_=pacp[:])
                            nc.sync.dma_start(out=drec[:], in_=rec[:])
                            nc.sync.dma_start(out=dpt[:], in_=pts[0][:])
                        if pend is not None:
                            finish_tail(pend)
                        pend = (anum, rec, h, b * 4 + qc)

                finish_tail(pend)
                pend = None
                # AllToAll for this head pass (pass-0 collective hides under pass 1)
                if h == 1:
                    for s in range(N_CORES):
                        nc.scalar.dma_start(out=attnt[0:64, s, :],
                                            in_=a2a_out[0][s])
                nc.gpsimd.collective_compute(
                    "AllToAll", ALU.bypass,
                    replica_groups=[list(range(N_CORES))],
                    ins=[a2a_in[h][:].opt()], outs=[a2a_out[h][:].opt()],
                )
                if h == 0:
                    # FFN weight prefetch (8MB fp8) rides the ACT hwdge queue so
                    # it never blocks the latency-critical sync-queue DMAs
                    nc.scalar.dma_start(w2t[:], w2[:])
                    nc.scalar.dma_start(w1t[:], w1[:])
                else:
                    for s in range(N_CORES):
                        nc.scalar.dma_start(out=attnt[64:128, s, :],
                                            in_=a2a_out[1][s])

            smp_cm.__exit__(None, None, None)
            ptp_cm.__exit__(None, None, None)
            attn_pool_cm.__exit__(None, None, None)

            if dbg:
                nc.sync.dma_start(dat[:], a2a_in[0][:])
                nc.sync.dma_start(dattnt[:], attnt[:])

            with tc.tile_pool(name="ef", bufs=1) as ef, \
                 tc.tile_pool(name="efw", bufs=4) as efw:
                # ===== Stage E: Wo (token-major out) + residual + LN2 =====
                x2 = ef.tile([128, 4, C], F32)
                h2t = ef.tile([128, KT, RS], F8)
                mv2 = ef.tile([128, 4, 2], F32)
                h2s = []
                for j in range(4):
                    for cc in range(2):
                        ps = ps_mm.tile([128, 512], F32, tag="psmm")
                        for t in range(KT):
                            nc.tensor.matmul(
                                ps[:],
                                lhsT=attnt[:, t, 128 * j:128 * (j + 1)],
                                rhs=wo_sb[:, t, 512 * cc:512 * (cc + 1)],
                                start=(t == 0), stop=(t == KT - 1))
                        nc.vector.tensor_tensor(
                            out=x2[:, j, 512 * cc:512 * (cc + 1)],
                            in0=xs_sb[:, j, 512 * cc:512 * (cc + 1)], in1=ps[:],
                            op=ALU.add)
                    stats2 = efw.tile([128, 2, 6], F32, tag="st2")
                    x2r = x2[:, j, :].rearrange("p (s f) -> p s f", f=512)
                    for s in range(2):
                        nc.vector.bn_stats(out=stats2[:, s, :], in_=x2r[:, s, :])
                    nc.vector.bn_aggr(out=mv2[:, j, :], in_=stats2[:])
                    rstd2 = emit_rsqrt(efw, mv2[:, j, 1:2], 1, "e")
                    h2 = efw.tile([128, C], BF16, tag="h2", bufs=4)
                    nc.vector.tensor_scalar(out=h2[:], in0=x2[:, j, :],
                                            scalar1=mv2[:, j, 0:1],
                                            scalar2=rstd2[:, 0:1],
                                            op0=ALU.subtract, op1=ALU.mult)
                    h2s.append(h2)
                for j in range(4):
                    for half in range(2):
                        ps_t = ps_mm.tile([128, 512], BF16, tag="psmm")
                        for k4 in range(4):
                            k = 4 * half + k4
                            nc.tensor.transpose(ps_t[:, 128 * k4:128 * (k4 + 1)],
                                                h2s[j][:, 128 * k:128 * (k + 1)],
                                                ident[:])
                        nc.vector.tensor_copy(
                            out=h2t[:, 4 * half:4 * half + 4,
                                    128 * j:128 * (j + 1)],
                            in_=ps_t[:].rearrange("p (a b) -> p a b", a=4))
                if dbg:
                    nc.sync.dma_start(dx2[:], x2[:])

                # =============== Stage F: FFN ===============
                hid = ef.tile([128, 32, RS], F8)
                for m in range(32):
                    ps = ps_mm.tile([128, 512], F32, tag="psmm")
                    for t in range(KT // 2):
                        nc.tensor.matmul(
                            ps[:],
                            lhsT=w1t[:, 2 * t:2 * t + 2, 128 * m:128 * (m + 1)],
                            rhs=h2t[:, 2 * t:2 * t + 2, :],
                            start=(t == 0), stop=(t == KT // 2 - 1), perf_mode=DR)
                    # hid = relu(ps/16 + b1) -- natural scale
                    nc.scalar.activation(out=hid[:, m, :], in_=ps[:], func=AF.Relu,
                                         bias=b1_sb[:, m:m + 1], scale=1.0 / 16.0)
                for j in range(4):
                    for cc in range(2):
                        ps = ps_mm.tile([128, 512], F32, tag="psmm")
                        for t in range(16):
                            nc.tensor.matmul(
                                ps[:],
                                lhsT=hid[:, 2 * t:2 * t + 2, 128 * j:128 * (j + 1)],
                                rhs=w2t[:, 2 * t:2 * t + 2, 512 * cc:512 * (cc + 1)],
                                start=(t == 0), stop=False, perf_mode=DR)
                        nc.tensor.matmul(
                            ps[:], lhsT=ones1x128[:],
                            rhs=b2row_sb[:, 512 * cc:512 * (cc + 1)],
                            start=False, stop=True)
                        yt = efw.tile([128, 512], F32, tag="yt")
                        nc.vector.tensor_tensor(
                            out=yt[:], in0=x2[:, j, 512 * cc:512 * (cc + 1)],
                            in1=ps[:], op=ALU.add)
                        nc.sync.dma_start(y[j][:, 512 * cc:512 * (cc + 1)], yt[:])

    nc.compile()
    return nc


def prep_inputs(x, Wq, Wk, Wv, Wo, bo, W1, b1, W2, b2, g1, be1, g2, be2):
    """Host-side sharding / layout prep. Returns list of per-core input dicts."""
    bf = ml_dtypes.bfloat16
    f8 = ml_dtypes.float8_e4m3
    x = np.asarray(x, np.float32).reshape(R, C)
    g1 = np.asarray(g1, np.float32); be1 = np.asarray(be1, np.float32)
    g2 = np.asarray(g2, np.float32); be2 = np.asarray(be2, np.float32)
    Wq = np.asarray(Wq, np.float32); Wk = np.asarray(Wk, np.float32)
    Wv = np.asarray(Wv, np.float32); Wo = np.asarray(Wo, np.float32)
    W1 = np.asarray(W1, np.float32); W2 = np.asarray(W2, np.float32)
    bo = np.asarray(bo, np.float32); b1 = np.asarray(b1, np.float32)
    b2 = np.asarray(b2, np.float32)

    Wq_f = g1[:, None] * Wq; bq_f = be1 @ Wq
    Wk_f = g1[:, None] * Wk; bk_f = be1 @ Wk
    Wv_f = g1[:, None] * Wv; bv_f = be1 @ Wv
    W1_f = g2[:, None] * W1; b1_f = b1 + be2 @ W1
    # the kernel emits no bias-add for q/k; v bias is folded through Wo into bo
    assert np.abs(bq_f).max() < 1e-6 and np.abs(bk_f).max() < 1e-6, \
        "kernel assumes zero folded q/k biases (be1 == 0)"
    bo_eff = bo + bv_f @ Wo

    def lhsT_layout(w, dt):  # [C_in, M] -> [128, C_in//128, M]
        ci, m = w.shape
        return np.ascontiguousarray(
            w.reshape(ci // 128, 128, m).transpose(1, 0, 2)).astype(dt)

    def bias_layout(v):  # [M] -> [128, M//128]
        return np.ascontiguousarray(v.reshape(-1, 128).T).astype(np.float32)

    x_bf_full = np.ascontiguousarray(
        x.reshape(8, 4, 128, C).transpose(0, 2, 1, 3).reshape(8, 128, 4 * C)
    ).astype(bf)
    wo_l = lhsT_layout(16.0 * Wo, bf)
    w1_l = lhsT_layout(16.0 * W1_f, f8)
    w2_l = lhsT_layout(16.0 * W2, f8)
    b1_l = bias_layout(b1_f)
    b2row = np.ascontiguousarray(16.0 * b2.reshape(1, C)).astype(bf)

    # causal partial-tile masks: mask[d][kl, ql] = 1 if 128*d + kl <= ql
    masks = np.zeros((4, 128, 512), np.float32)
    for d in range(4):
        kl = 128 * d + np.arange(128)[:, None]
        ql = np.arange(512)[None, :]
        masks[d] = (kl <= ql).astype(np.float32)
    masks = masks.astype(bf)

    ins = []
    for c in range(N_CORES):
        cs = slice(128 * c, 128 * (c + 1))
        ins.append({
            "x_bf": x_bf_full,
            "xs16": np.ascontiguousarray(
                (16.0 * (x[RS * c:RS * (c + 1)] + bo_eff[None, :])).reshape(4, 128, C)
            ).astype(np.float32),
            "wq": lhsT_layout(16.0 * Wq_f[:, cs], f8),
            "wk": lhsT_layout(16.0 * Wk_f[:, cs], f8),
            "wv": lhsT_layout(16.0 * Wv_f[:, cs], f8),
            "wo": wo_l,
            "w1": w1_l, "b1": b1_l,
            "w2": w2_l, "b2row": b2row,
            "masks": masks,
        })
    return ins


_NC_CACHE = {}


def kernel(**inputs):
    import time
    from concourse.bass_utils import run_bass_kernel_spmd
    if "nc" not in _NC_CACHE:
        _NC_CACHE["nc"] = build_nc()
    nc = _NC_CACHE["nc"]
    ins = prep_inputs(**inputs)
    res = None
    last_exc = None
    for _attempt in range(4):
        try:
            res = run_bass_kernel_spmd(nc, ins, core_ids=list(range(N_CORES)))
            break
        except Exception as e:  # transient device wedge (NRT_EXEC_UNIT_UNRECOVERABLE)
            last_exc = e
            time.sleep(2)
    if res is None:
        raise last_exc
    out = np.concatenate([r["y"].reshape(RS, C) for r in res.results], axis=0)
    return (out.reshape(B, T, C) * (1.0 / 16.0)).astype(np.float32)


# revision 40
# speedup vs baseline: 1.0762x; 1.0762x over previous
"""Trainium2 Bass kernel for a dense transformer block (B=2, T=2048, C=1024, H=16).

Sharding: tensor-parallel attention (2 heads/core, processed as two head
passes each followed by an AllToAll so the first collective hides under the
second pass) + row-parallel Wo/FFN (512 rows/core) across 8 NeuronCores.

Big GEMMs (QKV / Wo / W1 / W2) run fp8e4 with DoubleRow perf mode (2
k-rows/partition). Weights are pre-scaled by 16x on the host so their
uniform(-1/32,1/32) entries land in e4m3 normal range; the scale is undone
via the softmax exp scale, the V "ones" column, and a final /16 on the host
(every correction is a power of two, so exact).

Softmax denominators come free from PV matmuls via a constant column
appended to V; the reciprocal runs as a fast custom-DVE op and is broadcast
across partitions on the (otherwise idle) GPSIMD engine. LayerNorm rstd uses
exp(-0.5*ln(var+eps)) so the whole kernel needs a single activation table
set (natural_log_exp_and_others).
"""

import numpy as np
import ml_dtypes

import concourse.bass as bass
import concourse.bacc as bacc
import concourse.mybir as mybir
import concourse.tile as tile
from concourse.masks import make_identity


F32 = mybir.dt.float32
BF16 = mybir.dt.bfloat16
F8 = mybir.dt.float8e4
AF = mybir.ActivationFunctionType
ALU = mybir.AluOpType
DR = mybir.MatmulPerfMode.DoubleRow

N_CORES = 8
B, T, C, H, D, FF = 2, 2048, 1024, 16, 64, 4096
R = B * T            # 4096 total rows
RS = R // N_CORES    # 512 rows per core
KT = C // 128        # 8 k-tiles of the embedding dim
SCALE = 1.0 / np.sqrt(C)     # 2**-5 exact
EXP_SCALE = SCALE / 256.0    # undo the 16x on both Wq and Wk
LN_EPS = 1e-5
DEN_CONST = 16.0     # V "ones" column value -> pa[64] = 16*den -> rec = 1/(16*den)
OUT_SCALE = 1.0 / 16.0   # device output is 16*y; undone on the host


def build_nc():
    nc = bacc.Bacc(None, target_bir_lowering=False, debug=False, num_devices=N_CORES)

    # ---- per-core inputs (host pre-laid-out) ----
    x_bf = nc.dram_tensor("x_bf", [8, 128, 4 * C], BF16, kind="ExternalInput").ap()
    xs16 = nc.dram_tensor("xs16", [4, 128, C], F32, kind="ExternalInput").ap()
    wq = nc.dram_tensor("wq", [128, KT, 128], F8, kind="ExternalInput").ap()
    wk = nc.dram_tensor("wk", [128, KT, 128], F8, kind="ExternalInput").ap()
    wv = nc.dram_tensor("wv", [128, KT, 128], F8, kind="ExternalInput").ap()
    wo = nc.dram_tensor("wo", [128, KT, C], BF16, kind="ExternalInput").ap()
    w1 = nc.dram_tensor("w1", [128, KT, FF], F8, kind="ExternalInput").ap()
    b1 = nc.dram_tensor("b1", [128, 32], F32, kind="ExternalInput").ap()
    w2 = nc.dram_tensor("w2", [128, FF // 128, C], F8, kind="ExternalInput").ap()
    b2row = nc.dram_tensor("b2row", [1, C], BF16, kind="ExternalInput").ap()
    masks = nc.dram_tensor("masks", [4, 128, 512], BF16, kind="ExternalInput").ap()
    y = nc.dram_tensor("y", [4, 128, C], F32, kind="ExternalOutput").ap()
    import os
    dbg = os.environ.get("KDBG") == "1"
    if dbg:
        dq = nc.dram_tensor("dq", [128, R], BF16, kind="ExternalOutput").ap()
        dk = nc.dram_tensor("dk", [128, R], BF16, kind="ExternalOutput").ap()
        dv = nc.dram_tensor("dv", [128, R], BF16, kind="ExternalOutput").ap()
        dat = nc.dram_tensor("dat", [N_CORES, 64, RS], BF16, kind="ExternalOutput").ap()
        dattnt = nc.dram_tensor("dattnt", [128, KT, RS], BF16,
                                kind="ExternalOutput").ap()
        dx2 = nc.dram_tensor("dx2", [128, 4, C], F32, kind="ExternalOutput").ap()
        dh1 = nc.dram_tensor("dh1", [128, KT, 512], F8, kind="ExternalOutput").ap()
        dpa = nc.dram_tensor("dpa", [65, 512], F32, kind="ExternalOutput").ap()
        drec = nc.dram_tensor("drec", [1, 512], F32, kind="ExternalOutput").ap()
        drecb = nc.dram_tensor("drecb", [64, 512], F32, kind="ExternalOutput").ap()
        dpt = nc.dram_tensor("dpt", [128, 1024], BF16, kind="ExternalOutput").ap()

    with tile.TileContext(nc) as tc:
        with (
            tc.tile_pool(name="const", bufs=1) as const,
            tc.tile_pool(name="ps_mm", bufs=2, space="PSUM") as ps_mm,
            tc.tile_pool(name="ps_s", bufs=2, space="PSUM") as ps_s,
            tc.tile_pool(name="ps_a", bufs=2, space="PSUM") as ps_a,
            tc.tile_pool(name="dram", bufs=1, space="DRAM") as dram,
        ):
            # constants
            epst = const.tile([128, 1], F32)
            nc.any.memset(epst[:], LN_EPS)
            mask_sb = const.tile([128, 4, 512], BF16)
            for d in range(4):
                nc.scalar.dma_start(mask_sb[:, d, :], masks[d])
            wq_sb = const.tile([128, KT, 128], F8)
            nc.scalar.dma_start(wq_sb[:], wq[:])
            wk_sb = const.tile([128, KT, 128], F8)
            nc.scalar.dma_start(wk_sb[:], wk[:])
            wv_sb = const.tile([128, KT, 128], F8)
            nc.scalar.dma_start(wv_sb[:], wv[:])
            b1_sb = const.tile([128, 32], F32)
            nc.scalar.dma_start(b1_sb[:], b1[:])
            b2row_sb = const.tile([1, C], BF16)
            nc.scalar.dma_start(b2row_sb[:], b2row[:])
            ones1x128 = const.tile([1, 128], BF16)
            nc.any.memset(ones1x128[:], 1.0)
            ones64b = const.tile([1, 64], BF16)
            nc.any.memset(ones64b[:], 1.0)
            ident = const.tile([128, 128], BF16)
            make_identity(nc, ident[:])
            # persistent stage E/F tensors (prefetched at chosen points)
            wo_sb = const.tile([128, KT, C], BF16)
            w2t = const.tile([128, 32, C], F8)
            w1t = const.tile([128, KT, FF], F8)
            xs_sb = const.tile([128, 4, C], F32)
            attnt = const.tile([128, KT, RS], BF16)

            # prime the exp activation table set during warmup
            lnprime = const.tile([128, 1], F32)
            nc.scalar.activation(out=lnprime[:], in_=epst[:], func=AF.Exp)
            # magic constant for quake-rsqrt on the DVE (no ACT table needed)
            U32 = mybir.dt.uint32
            magic4 = const.tile([128, 4], F32)
            nc.any.memset(magic4[:].bitcast(U32), 0x5F3759DF)

            def emit_rsqrt(pool, var_ap, ncols, tagp):
                """rstd = 1/sqrt(var_ap + eps) via quake seed + 2 Newton steps."""
                ve = pool.tile([128, ncols], F32, tag=tagp + "ve")
                nc.vector.tensor_scalar(out=ve[:], in0=var_ap, scalar1=LN_EPS,
                                        scalar2=None, op0=ALU.add)
                y0 = pool.tile([128, ncols], F32, tag=tagp + "y0")
                nc.vector.tensor_scalar(out=y0[:].bitcast(U32),
                                        in0=ve[:].bitcast(U32), scalar1=1,
                                        scalar2=None,
                                        op0=ALU.logical_shift_right)
                nc.vector.tensor_tensor(out=y0[:].bitcast(U32),
                                        in0=magic4[:, 0:ncols].bitcast(U32),
                                        in1=y0[:].bitcast(U32), op=ALU.subtract)
                t = pool.tile([128, ncols], F32, tag=tagp + "tq")
                for _ in range(2):
                    nc.vector.tensor_tensor(out=t[:], in0=ve[:], in1=y0[:],
                                            op=ALU.mult)
                    nc.vector.tensor_tensor(out=t[:], in0=t[:], in1=y0[:],
                                            op=ALU.mult)
                    nc.vector.tensor_scalar(out=t[:], in0=t[:], scalar1=-0.5,
                                            scalar2=1.5, op0=ALU.mult,
                                            op1=ALU.add)
                    nc.vector.tensor_tensor(out=y0[:], in0=y0[:], in1=t[:],
                                            op=ALU.mult)
                return y0

            # HAM warmup: PE activity with no DMA dependency
            for wi in range(88):
                ps_w = ps_mm.tile([128, 512], F32, tag="psmm")
                nc.tensor.matmul(ps_w[:, 0:128], lhsT=ident[:], rhs=ident[:],
                                 start=True, stop=True)


            a2a_in = [dram.tile([N_CORES, 64, RS], BF16, name=f"a2ain{h}")
                      for h in range(2)]
            a2a_out = [dram.tile([N_CORES, 64, RS], BF16, name=f"a2aout{h}")
                       for h in range(2)]
            # early barrier: absorb per-core launch skew before the real work
            bar_in = dram.tile([N_CORES, 1, 16], F32, name="barin")
            bar_out = dram.tile([N_CORES, 1, 16], F32, name="barout")
            barsrc = const.tile([1, 16], F32)
            nc.any.memset(barsrc[:], 0.0)
            for s in range(N_CORES):
                nc.sync.dma_start(out=bar_in[s], in_=barsrc[:])
            nc.gpsimd.collective_compute(
                "AllToAll", ALU.bypass,
                replica_groups=[list(range(N_CORES))],
                ins=[bar_in[:].opt()], outs=[bar_out[:].opt()],
            )

            # attention-stage persistent tiles
            attn_pool_cm = tc.tile_pool(name="attn", bufs=1)
            attn = attn_pool_cm.__enter__()
            qt_sb = attn.tile([128, R], BF16)   # Q^T, feature-major (2 heads stacked)
            kt_sb = attn.tile([128, R], BF16)   # K^T
            vt_sb = attn.tile([128, R], BF16)   # V^T (pre-transpose)
            # V token-major, per 128-token chunk: [h0 d0..63 | 16.0 | h1 d0..63 | 16.0]
            v_sb = attn.tile([128, 32, 130], BF16)
            nc.vector.memset(v_sb[:, :, 64:65], DEN_CONST)
            nc.vector.memset(v_sb[:, :, 129:130], DEN_CONST)
            ptp_cm = tc.tile_pool(name="ptp", bufs=5)
            ptp = ptp_cm.__enter__()
            smp_cm = tc.tile_pool(name="smp", bufs=2)
            smp = smp_cm.__enter__()

            # ====== Stage A+B: LN1 + transpose + QKV per 512-row chunk ======
            with (
                tc.tile_pool(name="lnp", bufs=4) as lnp,
                tc.tile_pool(name="h1tp", bufs=2) as h1tp,
            ):
                for n in range(R // 512):
                    h1tn = h1tp.tile([128, KT, 512], F8, tag="h1tn")
                    xt4 = lnp.tile([128, 4 * C], BF16, tag="xt4", bufs=2)
                    qeng = nc.sync if n % 2 == 0 else nc.scalar
                    qeng.dma_start(xt4[:], x_bf[n])
                    mvn = lnp.tile([128, 4, 2], F32, tag="mvn")
                    xg = xt4[:].rearrange("p (i s f) -> p i s f", i=4, f=512)
                    for i4 in range(4):
                        stats = lnp.tile([128, 2, 6], F32, tag="st")
                        for s in range(2):
                            nc.vector.bn_stats(out=stats[:, s, :],
                                               in_=xg[:, i4, s, :])
                        nc.vector.bn_aggr(out=mvn[:, i4, :], in_=stats[:])
                    rstd4 = emit_rsqrt(lnp, mvn[:, :, 1], 4, "a")
                    for i4 in range(4):
                        hn = lnp.tile([128, C], BF16, tag="hn")
                        nc.vector.tensor_scalar(out=hn[:],
                                                in0=xt4[:, C * i4:C * (i4 + 1)],
                                                scalar1=mvn[:, i4, 0:1],
                                                scalar2=rstd4[:, i4:i4 + 1],
                                                op0=ALU.subtract, op1=ALU.mult)
                        for half in range(2):
                            ps_t = ps_mm.tile([128, 512], BF16, tag="psmm")
                            for j4 in range(4):
                                j = 4 * half + j4
                                nc.tensor.transpose(ps_t[:, 128 * j4:128 * (j4 + 1)],
                                                    hn[:, 128 * j:128 * (j + 1)],
                                                    ident[:])
                            nc.vector.tensor_copy(
                                out=h1tn[:, 4 * half:4 * half + 4,
                                         128 * i4:128 * (i4 + 1)],
                                in_=ps_t[:].rearrange("p (a b) -> p a b", a=4))
                    for w_sb, out_sb in ((wq_sb, qt_sb), (wk_sb, kt_sb),
                                         (wv_sb, vt_sb)):
                        ps = ps_mm.tile([128, 512], F32, tag="psmm")
                        for t in range(KT // 2):
                            nc.tensor.matmul(ps[:], lhsT=w_sb[:, 2 * t:2 * t + 2, :],
                                             rhs=h1tn[:, 2 * t:2 * t + 2, :],
                                             start=(t == 0), stop=(t == KT // 2 - 1),
                                             perf_mode=DR)
                        nc.scalar.copy(out=out_sb[:, 512 * n:512 * (n + 1)],
                                       in_=ps[:])
                    if dbg and n == 0:
                        nc.sync.dma_start(dh1[:], h1tn[:])
                    # V token-major for this 512-token group (PE transposes)
                    g = n
                    ps_t = ps_mm.tile([128, 512], BF16, tag="psmm")
                    for j4 in range(4):
                        j = 4 * g + j4
                        nc.tensor.transpose(ps_t[:, 128 * j4:128 * (j4 + 1)],
                                            vt_sb[:, 128 * j:128 * (j + 1)], ident[:])
                    pst3 = ps_t[:].rearrange("p (a b) -> p a b", a=4)
                    nc.scalar.copy(out=v_sb[:, 4 * g:4 * g + 4, 0:64],
                                   in_=pst3[:, :, 0:64])
                    nc.scalar.copy(out=v_sb[:, 4 * g:4 * g + 4, 65:129],
                                   in_=pst3[:, :, 64:128])

            # stage-E inputs (xs16 2MB + wo 2MB) now that x has drained
            for j in range(4):
                nc.scalar.dma_start(xs_sb[:, j, :], xs16[j])
            nc.scalar.dma_start(wo_sb[:], wo[:])
            if dbg:
                nc.sync.dma_start(dq[:], qt_sb[:])
                nc.sync.dma_start(dk[:], kt_sb[:])
                nc.sync.dma_start(dv[:], vt_sb[:])

            # =============== Stage C: attention, one head per pass ===============
            def finish_tail(pend):
                anum, rec, hh, shard = pend
                pb = ps_mm.tile([64, 512], F32, tag="psmm")
                nc.tensor.matmul(pb[:], lhsT=ones64b[:], rhs=rec[:],
                                 start=True, stop=True)
                at16 = smp.tile([64, 512], BF16, tag="at16")
                with nc.allow_low_precision(reason="attn out bf16"):
                    nc.vector.tensor_tensor(out=at16[:], in0=anum[:], in1=pb[:],
                                            op=ALU.mult)
                nc.sync.dma_start(out=a2a_in[hh][shard], in_=at16[:])

            pend = None
            for h in range(2):
                hp = 64 * h
                vo = 65 * h
                for b in range(B):
                    for qc in range(4):
                        q0 = b * T + 512 * qc
                        nkt = 4 * (qc + 1)
                        npair = nkt // 2
                        pts = {}
                        pa = ps_a.tile([65, 512], F32, tag="pa")

                        def emit_qk(p):
                            ps = ps_s.tile([128, 1024], F32, tag="pss")
                            for u in range(2):
                                k = 2 * p + u
                                nc.tensor.matmul(
                                    ps[:, 512 * u:512 * (u + 1)],
                                    lhsT=kt_sb[hp:hp + 64,
                                               b * T + 128 * k:b * T + 128 * (k + 1)],
                                    rhs=qt_sb[hp:hp + 64, q0:q0 + 512],
                                    start=True, stop=True, tile_position=(hp, 0))
                            pt = ptp.tile([128, 1024], BF16, tag="pt")
                            nc.scalar.activation(out=pt[:], in_=ps[:],
                                                 func=AF.Exp, scale=EXP_SCALE)
                            for u in range(2):
                                k = 2 * p + u
                                if k >= 4 * qc:
                                    sl = pt[:, 512 * u:512 * (u + 1)]
                                    nc.vector.tensor_tensor(
                                        out=sl, in0=sl,
                                        in1=mask_sb[:, k - 4 * qc, :], op=ALU.mult)
                            pts[p] = pt

                        def emit_pv(p):
                            for u in range(2):
                                k = 2 * p + u
                                nc.tensor.matmul(
                                    pa[:],
                                    lhsT=v_sb[:, b * 16 + k, vo:vo + 65],
                                    rhs=pts[p][:, 512 * u:512 * (u + 1)],
                                    start=(k == 0), stop=(k == nkt - 1),
                                    tile_position=(0, 0))

                        for p in range(npair + 1):
                            if p < npair:
                                emit_qk(p)
                            if p >= 1:
                                emit_pv(p - 1)
                        # softmax tail: anum + den to SBUF (frees pa),
                        # rec = 1/(16*den); broadcast matmul deferred one chunk
                        anum = smp.tile([64, 512], BF16, tag="anum")
                        with nc.allow_low_precision(reason="attn numerator bf16"):
                            nc.vector.tensor_copy(out=anum[:], in_=pa[0:64, :])
                        dcp = smp.tile([1, 512], F32, tag="dcp")
                        nc.scalar.copy(out=dcp[:], in_=pa[64:65, :])
                        rec32 = smp.tile([1, 512], F32, tag="rec32")
                        rscr = smp.tile([1, 512], F32, tag="rscr")
                        nc.vector.reciprocal_approx_accurate(out=rec32[:],
                                                             in_=dcp[:],
                                                             scratch=rscr[:])
                        rec = smp.tile([1, 512], BF16, tag="rec")
                        with nc.allow_low_precision(reason="softmax denom bf16"):
                            nc.vector.tensor_copy(out=rec[:], in_=rec32[:])
                        if dbg and h == 0 and b == 0 and qc == 0:
                            pacp = smp.tile([65, 512], F32, tag="pacp")
                            nc.scalar.copy(out=pacp[:], in_=pa[:])
                            nc.sync.dma_start(out=dpa[:], in_=pacp[:])
                            nc.sync.dma_start(out=drec[:], in_=rec[:])
                            nc.sync.dma_start(out=dpt[:], in_=pts[0][:])
                        if pend is not None:
                            finish_tail(pend)
                        pend = (anum, rec, h, b * 4 + qc)

                finish_tail(pend)
                pend = None
                # AllToAll for this head pass (pass-0 collective hides under pass 1)
                if h == 1:
                    for s in range(N_CORES):
                        nc.scalar.dma_start(out=attnt[0:64, s, :],
                                            in_=a2a_out[0][s])
                nc.gpsimd.collective_compute(
                    "AllToAll", ALU.bypass,
                    replica_groups=[list(range(N_CORES))],
                    ins=[a2a_in[h][:].opt()], outs=[a2a_out[h][:].opt()],
                )
                if h == 0:
                    # FFN weight prefetch (8MB fp8) rides the ACT hwdge queue so
                    # it never blocks the latency-critical sync-queue DMAs
                    nc.scalar.dma_start(w2t[:], w2[:])
                    nc.scalar.dma_start(w1t[:], w1[:])
                else:
                    for s in range(N_CORES):
                        nc.scalar.dma_start(out=attnt[64:128, s, :],
                                            in_=a2a_out[1][s])

            smp_cm.__exit__(None, None, None)
            ptp_cm.__exit__(None, None, None)
            attn_pool_cm.__exit__(None, None, None)

            if dbg:
                nc.sync.dma_start(dat[:], a2a_in[0][:])
                nc.sync.dma_start(dattnt[:], attnt[:])

            with tc.tile_pool(name="ef", bufs=1) as ef, \
                 tc.tile_pool(name="efw", bufs=4) as efw:
                # ===== Stage E: Wo (token-major out) + residual + LN2 =====
                x2 = ef.tile([128, 4, C], F32)
                h2t = ef.tile([128, KT, RS], F8)
                mv2 = ef.tile([128, 4, 2], F32)
                h2s = []
                for j in range(4):
                    for cc in range(2):
                        ps = ps_mm.tile([128, 512], F32, tag="psmm")
                        for t in range(KT):
                            nc.tensor.matmul(
                                ps[:],
                                lhsT=attnt[:, t, 128 * j:128 * (j + 1)],
                                rhs=wo_sb[:, t, 512 * cc:512 * (cc + 1)],
                                start=(t == 0), stop=(t == KT - 1))
                        nc.vector.tensor_tensor(
                            out=x2[:, j, 512 * cc:512 * (cc + 1)],
                            in0=xs_sb[:, j, 512 * cc:512 * (cc + 1)], in1=ps[:],
                            op=ALU.add)
                    stats2 = efw.tile([128, 2, 6], F32, tag="st2")
                    x2r = x2[:, j, :].rearrange("p (s f) -> p s f", f=512)
                    for s in range(2):
                        nc.vector.bn_stats(out=stats2[:, s, :], in_=x2r[:, s, :])
                    nc.vector.bn_aggr(out=mv2[:, j, :], in_=stats2[:])
                    rstd2 = emit_rsqrt(efw, mv2[:, j, 1:2], 1, "e")
                    h2 = efw.tile([128, C], BF16, tag="h2", bufs=4)
                    nc.vector.tensor_scalar(out=h2[:], in0=x2[:, j, :],
                                            scalar1=mv2[:, j, 0:1],
                                            scalar2=rstd2[:, 0:1],
                                            op0=ALU.subtract, op1=ALU.mult)
                    h2s.append(h2)
                for j in range(4):
                    for half in range(2):
                        ps_t = ps_mm.tile([128, 512], BF16, tag="psmm")
                        for k4 in range(4):
                            k = 4 * half + k4
                            nc.tensor.transpose(ps_t[:, 128 * k4:128 * (k4 + 1)],
                                                h2s[j][:, 128 * k:128 * (k + 1)],
                                                ident[:])
                        nc.vector.tensor_copy(
                            out=h2t[:, 4 * half:4 * half + 4,
                                    128 * j:128 * (j + 1)],
                            in_=ps_t[:].rearrange("p (a b) -> p a b", a=4))
                if dbg:
                    nc.sync.dma_start(dx2[:], x2[:])

                # =============== Stage F: FFN ===============
                hid = ef.tile([128, 32, RS], F8)
                for m in range(32):
                    ps = ps_mm.tile([128, 512], F32, tag="psmm")
                    for t in range(KT // 2):
                        nc.tensor.matmul(
                            ps[:],
                            lhsT=w1t[:, 2 * t:2 * t + 2, 128 * m:128 * (m + 1)],
                            rhs=h2t[:, 2 * t:2 * t + 2, :],
                            start=(t == 0), stop=(t == KT // 2 - 1), perf_mode=DR)
                    # hid = relu(ps/16 + b1) -- natural scale
                    nc.scalar.activation(out=hid[:, m, :], in_=ps[:], func=AF.Relu,
                                         bias=b1_sb[:, m:m + 1], scale=1.0 / 16.0)
                for j in range(4):
                    for cc in range(2):
                        ps = ps_mm.tile([128, 512], F32, tag="psmm")
                        for t in range(16):
                            nc.tensor.matmul(
                                ps[:],
                                lhsT=hid[:, 2 * t:2 * t + 2, 128 * j:128 * (j + 1)],
                                rhs=w2t[:, 2 * t:2 * t + 2, 512 * cc:512 * (cc + 1)],
                                start=(t == 0), stop=False, perf_mode=DR)
                        nc.tensor.matmul(
                            ps[:], lhsT=ones1x128[:],
                            rhs=b2row_sb[:, 512 * cc:512 * (cc + 1)],
                            start=False, stop=True)
                        yt = efw.tile([128, 512], F32, tag="yt")
                        nc.vector.tensor_tensor(
                            out=yt[:], in0=x2[:, j, 512 * cc:512 * (cc + 1)],
                            in1=ps[:], op=ALU.add)
                        nc.sync.dma_start(y[j][:, 512 * cc:512 * (cc + 1)], yt[:])

    nc.compile()
    return nc


def prep_inputs(x, Wq, Wk, Wv, Wo, bo, W1, b1, W2, b2, g1, be1, g2, be2):
    """Host-side sharding / layout prep. Returns list of per-core input dicts."""
    bf = ml_dtypes.bfloat16
    f8 = ml_dtypes.float8_e4m3
    x = np.asarray(x, np.float32).reshape(R, C)
    g1 = np.asarray(g1, np.float32); be1 = np.asarray(be1, np.float32)
    g2 = np.asarray(g2, np.float32); be2 = np.asarray(be2, np.float32)
    Wq = np.asarray(Wq, np.float32); Wk = np.asarray(Wk, np.float32)
    Wv = np.asarray(Wv, np.float32); Wo = np.asarray(Wo, np.float32)
    W1 = np.asarray(W1, np.float32); W2 = np.asarray(W2, np.float32)
    bo = np.asarray(bo, np.float32); b1 = np.asarray(b1, np.float32)
    b2 = np.asarray(b2, np.float32)

    Wq_f = g1[:, None] * Wq; bq_f = be1 @ Wq
    Wk_f = g1[:, None] * Wk; bk_f = be1 @ Wk
    Wv_f = g1[:, None] * Wv; bv_f = be1 @ Wv
    W1_f = g2[:, None] * W1; b1_f = b1 + be2 @ W1
    # the kernel emits no bias-add for q/k; v bias is folded through Wo into bo
    assert np.abs(bq_f).max() < 1e-6 and np.abs(bk_f).max() < 1e-6, \
        "kernel assumes zero folded q/k biases (be1 == 0)"
    bo_eff = bo + bv_f @ Wo

    def lhsT_layout(w, dt):  # [C_in, M] -> [128, C_in//128, M]
        ci, m = w.shape
        return np.ascontiguousarray(
            w.reshape(ci // 128, 128, m).transpose(1, 0, 2)).astype(dt)

    def bias_layout(v):  # [M] -> [128, M//128]
        return np.ascontiguousarray(v.reshape(-1, 128).T).astype(np.float32)

    x_bf_full = np.ascontiguousarray(
        x.reshape(8, 4, 128, C).transpose(0, 2, 1, 3).reshape(8, 128, 4 * C)
    ).astype(bf)
    wo_l = lhsT_layout(16.0 * Wo, bf)
    w1_l = lhsT_layout(16.0 * W1_f, f8)
    w2_l = lhsT_layout(16.0 * W2, f8)
    b1_l = bias_layout(b1_f)
    b2row = np.ascontiguousarray(16.0 * b2.reshape(1, C)).astype(bf)

    # causal partial-tile masks: mask[d][kl, ql] = 1 if 128*d + kl <= ql
    masks = np.zeros((4, 128, 512), np.float32)
    for d in range(4):
        kl = 128 * d + np.arange(128)[:, None]
        ql = np.arange(512)[None, :]
        masks[d] = (kl <= ql).astype(np.float32)
    masks = masks.astype(bf)

    ins = []
    for c in range(N_CORES):
        cs = slice(128 * c, 128 * (c + 1))
        ins.append({
            "x_bf": x_bf_full,
            "xs16": np.ascontiguousarray(
                (16.0 * (x[RS * c:RS * (c + 1)] + bo_eff[None, :])).reshape(4, 128, C)
            ).astype(np.float32),
            "wq": lhsT_layout(16.0 * Wq_f[:, cs], f8),
            "wk": lhsT_layout(16.0 * Wk_f[:, cs], f8),
            "wv": lhsT_layout(16.0 * Wv_f[:, cs], f8),
            "wo": wo_l,
            "w1": w1_l, "b1": b1_l,
            "w2": w2_l, "b2row": b2row,
            "masks": masks,
        })
    return ins


_NC_CACHE = {}


def kernel(**inputs):
    import time
    from concourse.bass_utils import run_bass_kernel_spmd
    if "nc" not in _NC_CACHE:
        _NC_CACHE["nc"] = build_nc()
    nc = _NC_CACHE["nc"]
    ins = prep_inputs(**inputs)
    res = None
    last_exc = None
    for _attempt in range(4):
        try:
            res = run_bass_kernel_spmd(nc, ins, core_ids=list(range(N_CORES)))
            break
        except Exception as e:  # transient device wedge (NRT_EXEC_UNIT_UNRECOVERABLE)
            last_exc = e
            time.sleep(2)
    if res is None:
        raise last_exc
    out = np.concatenate([r["y"].reshape(RS, C) for r in res.results], axis=0)
    return (out.reshape(B, T, C) * (1.0 / 16.0)).astype(np.float32)
